# revision 1
# baseline (speedup 1.0000x reference)
"""Grouped linear (MoE routing) kernel for 8 Trainium2 NeuronCores.

out[t] = input_tokens[t] @ weight[expert_assignments[t]].T

Strategy (expert-parallel): the host groups tokens by expert (argsort),
pads every group to a common capacity C (multiple of 128), and core e
computes the dense GEMM  Y_e = X_e @ W_e.T  for expert e.  The host then
scatters rows back to the original token order.

Per-core Bass/Tile kernel: X is staged in DRAM pre-transposed ([in, C])
so the contraction dim lands on SBUF partitions; W is staged as W.T
([in, out]).  The full W.T (16 MB) stays resident in SBUF, loaded as 4
column blocks (4 MB each, rings alternated) so the PE can start ~12 us
in instead of waiting for the full 16 MB.  The first A=3 token tiles are
processed column-block-major ("phase A") to cover the tail of the W
load; the rest run token-major with one full-width output DMA per token
tile.  Matmuls run in float32r (fp32 data, reduced-precision single-pass
multiply) which streams at the full PE rate; PSUM eviction is pinned to
the Vector engine.
"""

import numpy as np

import concourse.mybir as mybir
import concourse.tile as tile
from concourse import bacc
from concourse.bass_utils import run_bass_kernel_spmd

NUM_EXPERTS = 8
D_IN = 2048
D_OUT = 2048
P = 128
KO = D_IN // P      # 16 contraction subtiles
NBLK = 512          # psum bank width (fp32)
NB = D_OUT // NBLK  # 4 output column blocks

MM_DT = mybir.dt.float32r
PHASE_A = 3      # token tiles processed column-block-major during the W load
X_BUFS = 3
O_BUFS = 3
COPY_ENG = "vector"

_nc_cache = {}


def _build_nc(C: int, reps: int = 1, reload_w: bool = False):
    """Bass module: y[C, D_OUT] = xT.T @ wT  (xT: [D_IN, C], wT: [D_IN, D_OUT]).

    reps > 1 appends extra full sweeps inside the NEFF (timing calibration
    only -- the slope of wall time vs reps isolates device time).  With
    reload_w each sweep re-issues the W DMAs into the same tiles, so WAR
    deps serialize sweeps and the slope includes the one-shot W prologue.
    """
    nc = bacc.Bacc("TRN2", target_bir_lowering=False, debug=False,
                   num_devices=NUM_EXPERTS)
    xT = nc.dram_tensor("xT", [D_IN, C], MM_DT, kind="ExternalInput")
    wT = nc.dram_tensor("wT", [D_IN, D_OUT], MM_DT, kind="ExternalInput")
    y = nc.dram_tensor("y", [C, D_OUT], mybir.dt.float32, kind="ExternalOutput")

    M_TILES = C // P
    A = min(PHASE_A, M_TILES)  # phase-A token tiles (overlap the W block load)
    xT3 = xT.rearrange("(ko p) m -> p ko m", p=P)
    wT3 = wT.rearrange("(ko p) n -> p ko n", p=P)

    with tile.TileContext(nc) as tc:
        with (
            tc.tile_pool(name="w", bufs=1) as wpool,
            tc.tile_pool(name="x", bufs=X_BUFS) as xpool,
            tc.tile_pool(name="oa", bufs=2 if A else 1) as oapool,
            tc.tile_pool(name="ob", bufs=O_BUFS) as obpool,
            tc.tile_pool(name="ps", bufs=8, space="PSUM") as pspool,
        ):
            w_tiles = [
                wpool.tile([P, KO, NBLK], MM_DT, tag=f"w{nb}", name=f"w{nb}")
                for nb in range(NB)
            ]

            def mm_group(m, xt, nb):
                ps = pspool.tile([P, NBLK], mybir.dt.float32)
                for ks in range(KO):
                    nc.tensor.matmul(
                        ps[:],
                        lhsT=xt[:, ks, :],
                        rhs=w_tiles[nb][:, ks, :],
                        start=(ks == 0),
                        stop=(ks == KO - 1),
                    )
                return ps

            def body(load_w):
                # Phase-A X tiles ride the ACT ring so they aren't queued
                # behind W on the SP ring (HWDGE is FIFO per ring); W column
                # blocks alternate rings so block arrivals interleave.
                xa_tiles = []
                for m in range(A):
                    xt = xpool.tile([P, KO, P], MM_DT, tag="x", name=f"xa{m}")
                    nc.scalar.dma_start(xt[:], xT3[:, :, m * P:(m + 1) * P])
                    xa_tiles.append(xt)

                if load_w:
                    for nb in range(NB):
                        eng = nc.scalar if nb % 2 == 1 else nc.sync
                        eng.dma_start(
                            w_tiles[nb][:], wT3[:, :, nb * NBLK:(nb + 1) * NBLK])

                # Phase A: column-block-major over the first A token tiles,
                # small per-block outputs on the ACT ring.
                for nb in range(NB):
                    for m in range(A):
                        ps = mm_group(m, xa_tiles[m], nb)
                        ot = oapool.tile([P, NBLK], mybir.dt.float32)
                        nc.vector.tensor_copy(out=ot[:], in_=ps[:])
                        nc.scalar.dma_start(
                            y[m * P:(m + 1) * P, nb * NBLK:(nb + 1) * NBLK],
                            ot[:])

                # Phase B: token-major, one full-width output DMA per tile
                # on the SP ring.
                for m in range(A, M_TILES):
                    xt = xpool.tile([P, KO, P], MM_DT, tag="x", name=f"xb{m}")
                    nc.sync.dma_start(xt[:], xT3[:, :, m * P:(m + 1) * P])
                    ot = obpool.tile([P, D_OUT], mybir.dt.float32)
                    for nb in range(NB):
                        ps = mm_group(m, xt, nb)
                        dst = ot[:, nb * NBLK:(nb + 1) * NBLK]
                        if COPY_ENG == "vector":
                            nc.vector.tensor_copy(out=dst, in_=ps[:])
                        else:
                            nc.any.tensor_copy(out=dst, in_=ps[:])
                    nc.sync.dma_start(y[m * P:(m + 1) * P, :], ot[:])

            body(load_w=True)
            for _ in range(1, reps):
                body(load_w=reload_w)

    nc.compile()
    return nc


def _get_nc(C: int):
    if C not in _nc_cache:
        _nc_cache[C] = _build_nc(C)
    return _nc_cache[C]


def _route(input_tokens, expert_assignments):
    """Host-side dispatch: group tokens by expert, pad to capacity."""
    a = np.asarray(expert_assignments)
    x = np.ascontiguousarray(np.asarray(input_tokens, dtype=np.float32))
    order = np.argsort(a, kind="stable")
    counts = np.bincount(a.astype(np.int64), minlength=NUM_EXPERTS)
    starts = np.zeros(NUM_EXPERTS + 1, dtype=np.int64)
    np.cumsum(counts, out=starts[1:])
    C = max(P, int(-(-counts.max() // P)) * P)
    xs = x[order]  # [T, D_IN] sorted by expert
    xsT = np.ascontiguousarray(xs.T)  # [D_IN, T]
    return order, counts, starts, C, xsT


def kernel(input_tokens, weight, expert_assignments):
    order, counts, starts, C, xsT = _route(input_tokens, expert_assignments)
    w = np.asarray(weight, dtype=np.float32)
    T = xsT.shape[1]

    nc = _get_nc(C)
    in_maps = []
    for e in range(NUM_EXPERTS):
        s, cnt = int(starts[e]), int(counts[e])
        xTe = np.zeros((D_IN, C), dtype=np.float32)
        xTe[:, :cnt] = xsT[:, s:s + cnt]
        wTe = np.ascontiguousarray(w[e].T)  # [in, out]
        in_maps.append({"xT": xTe, "wT": wTe})

    res = run_bass_kernel_spmd(nc, in_maps, list(range(NUM_EXPERTS)))

    out = np.empty((T, D_OUT), dtype=np.float32)
    for e in range(NUM_EXPERTS):
        s, cnt = int(starts[e]), int(counts[e])
        out[order[s:s + cnt]] = res.results[e]["y"][:cnt]
    return out

